# revision 1
# baseline (speedup 1.0000x reference)
import functools
import numpy as np
import jax
import jax.numpy as jnp

# nn_CNFVectorModule: equivariant GNN message passing, N=1024 particles.
# Sharding: pairwise i-axis row-parallel across 8 NeuronCores (each core owns
# 128 rows i of r_ij/rbf/filters; feature dicts replicated, all-gathered
# between layers so the j-contraction is fully local).

N = 1024
NCORES = 8
LOCAL = N // NCORES
R_CUT = 5.0
EPS = 1e-7


def _swish(x):
    return x * jax.nn.sigmoid(x)


def _mlp(x, p):
    n = len(p["W"])
    for i in range(n):
        x = x @ p["W"][i] + p["b"][i]
        if i < n - 1:
            x = _swish(x)
    return x


def _filter(rbf, p, mask):
    return _mlp(rbf, p) * mask[..., None]


def _shard_forward(pos_l, mask_l, t, y, f0, params):
    # pos_l: (LOCAL,3) this core's i-rows.  mask_l: (LOCAL,N) ~eye slice.
    positions = y[0]
    r_ij = pos_l[:, None, :] - positions[None, :, :]          # (LOCAL,N,3)
    norms = jnp.sqrt(jnp.sum(r_ij * r_ij, axis=-1))           # (LOCAL,N)
    unit = r_ij / (norms[..., None] + EPS)
    norms = jnp.where(mask_l, norms, 0.0)
    freqs = jnp.arange(1, 33, dtype=jnp.float32)
    rbf = jnp.sqrt(2.0 / R_CUT) * jnp.sin(
        freqs * jnp.pi * norms[..., None] / R_CUT) / (norms[..., None] + EPS)
    rbf = jnp.where(mask_l[..., None], rbf, 0.0)              # (LOCAL,N,32)

    t_emb = _mlp(jnp.reshape(t, (1,)), params["time"])
    t_feat = jnp.broadcast_to(t_emb[None, :, None], (N, 8, 1))
    L0 = jnp.concatenate([f0, t_feat], axis=1)                # (N,16,1)
    L1 = None

    for lp in params["layers"]:
        R00 = _filter(rbf, lp["f00"], mask_l)
        out0 = jnp.einsum("ijc,jck->ick", R00, L0)
        R01 = _filter(rbf, lp["f01"], mask_l)
        out1 = jnp.einsum("ijc,ijm,jc->icm", R01, unit, L0[..., 0])
        if L1 is not None:
            R10 = _filter(rbf, lp["f10"], mask_l)
            p10 = jnp.einsum("ijc,ijm,jcm->ic", R10, unit, L1)[..., None]
            R11 = _filter(rbf, lp["f11"], mask_l)
            p11 = jnp.einsum("ijc,jcm->icm", R11, L1)
            out0 = jnp.concatenate([out0, p10], axis=1)
            out1 = jnp.concatenate([out1, p11], axis=1)
        # gather local-i conv outputs -> full N so the per-particle MLPs and
        # the next layer's j-contraction see every particle (tiny: ~256KB)
        out0 = jax.lax.all_gather(out0, "i").reshape(N, *out0.shape[1:])
        out1 = jax.lax.all_gather(out1, "i").reshape(N, *out1.shape[1:])
        L0 = _mlp(out0[..., 0], lp["mlp0"])[..., None]
        vm = jnp.einsum("ncm,cd->ndm", out1, lp["mix1"])
        vnorm = jnp.sqrt(jnp.sum(vm * vm, axis=-1) + EPS)
        gate = jax.nn.sigmoid(_mlp(vnorm, lp["gate1"]))
        L1 = vm * gate[..., None]

    scales = jnp.repeat(L0[:, 0, :], repeats=3, axis=-1)      # (N,3)
    translations = L1[:, 1, :]
    velocities = y[1]
    ydot = jnp.stack([velocities, scales * velocities + translations])
    return ydot, scales, translations


@functools.partial(jax.pmap, axis_name="i",
                   in_axes=(0, 0, None, None, None, None),
                   devices=jax.devices()[:NCORES])
def _pmapped(pos_l, mask_l, t, y, f0, params):
    return _shard_forward(pos_l, mask_l, t, y, f0, params)


def kernel(t, y, f0, params):
    t = np.asarray(t, np.float32)
    y = np.asarray(y, np.float32)
    f0 = np.asarray(f0, np.float32)
    pos = y[0]
    pos_sh = pos.reshape(NCORES, LOCAL, 3)
    mask = ~np.eye(N, dtype=bool)
    mask_sh = mask.reshape(NCORES, LOCAL, N)
    ydot, scales, translations = _pmapped(pos_sh, mask_sh, t, y, f0, params)
    # every device holds the identical full-shape result; take core 0's copy
    return (np.asarray(ydot[0]), np.asarray(scales[0]),
            np.asarray(translations[0]))


# revision 4
# speedup vs baseline: 1.0788x; 1.0788x over previous
import functools
import numpy as np
import jax
import jax.numpy as jnp

# nn_CNFVectorModule: equivariant GNN message passing, N=1024 particles.
# Sharding: pairwise i-axis row-parallel across 8 NeuronCores (each core owns
# 128 rows i of r_ij/rbf/filters; feature dicts replicated, all-gathered
# between layers so the j-contraction is fully local).

N = 1024
NCORES = 8
LOCAL = N // NCORES
R_CUT = 5.0
EPS = 1e-7


def _swish(x):
    return x * jax.nn.sigmoid(x)


def _mlp(x, p):
    n = len(p["W"])
    for i in range(n):
        x = x @ p["W"][i] + p["b"][i]
        if i < n - 1:
            x = _swish(x)
    return x


def _filter(rbf, p, mask):
    return _mlp(rbf, p) * mask[..., None]


def _shard_forward(pos_l, t, y, f0, params):
    # pos_l: (LOCAL,3) this core's i-rows; mask = ~eye rows for this shard
    gi = jax.lax.axis_index("i") * LOCAL + jnp.arange(LOCAL)
    mask_l = gi[:, None] != jnp.arange(N)[None, :]
    positions = y[0]
    r_ij = pos_l[:, None, :] - positions[None, :, :]          # (LOCAL,N,3)
    norms = jnp.sqrt(jnp.sum(r_ij * r_ij, axis=-1))           # (LOCAL,N)
    unit = r_ij / (norms[..., None] + EPS)
    norms = jnp.where(mask_l, norms, 0.0)
    freqs = jnp.arange(1, 33, dtype=jnp.float32)
    rbf = jnp.sqrt(2.0 / R_CUT) * jnp.sin(
        freqs * jnp.pi * norms[..., None] / R_CUT) / (norms[..., None] + EPS)
    rbf = jnp.where(mask_l[..., None], rbf, 0.0)              # (LOCAL,N,32)

    t_emb = _mlp(jnp.reshape(t, (1,)), params["time"])
    t_feat = jnp.broadcast_to(t_emb[None, :, None], (N, 8, 1))
    L0 = jnp.concatenate([f0, t_feat], axis=1)                # (N,16,1)
    L1 = None

    for lp in params["layers"]:
        R00 = _filter(rbf, lp["f00"], mask_l)
        out0 = jnp.einsum("ijc,jck->ick", R00, L0)
        R01 = _filter(rbf, lp["f01"], mask_l)
        out1 = jnp.einsum("ijc,ijm,jc->icm", R01, unit, L0[..., 0])
        if L1 is not None:
            R10 = _filter(rbf, lp["f10"], mask_l)
            p10 = jnp.einsum("ijc,ijm,jcm->ic", R10, unit, L1)[..., None]
            R11 = _filter(rbf, lp["f11"], mask_l)
            p11 = jnp.einsum("ijc,jcm->icm", R11, L1)
            out0 = jnp.concatenate([out0, p10], axis=1)
            out1 = jnp.concatenate([out1, p11], axis=1)
        # gather local-i conv outputs -> full N so the per-particle MLPs and
        # the next layer's j-contraction see every particle (tiny: ~256KB)
        out0 = jax.lax.all_gather(out0, "i").reshape(N, *out0.shape[1:])
        out1 = jax.lax.all_gather(out1, "i").reshape(N, *out1.shape[1:])
        L0 = _mlp(out0[..., 0], lp["mlp0"])[..., None]
        vm = jnp.einsum("ncm,cd->ndm", out1, lp["mix1"])
        vnorm = jnp.sqrt(jnp.sum(vm * vm, axis=-1) + EPS)
        gate = jax.nn.sigmoid(_mlp(vnorm, lp["gate1"]))
        L1 = vm * gate[..., None]

    scales = jnp.repeat(L0[:, 0, :], repeats=3, axis=-1)      # (N,3)
    translations = L1[:, 1, :]
    velocities = y[1]
    ydot = jnp.stack([velocities, scales * velocities + translations])
    return ydot, scales, translations


@functools.partial(jax.pmap, axis_name="i",
                   in_axes=(0, None, None, None, None),
                   devices=jax.devices()[:NCORES])
def _pmapped(pos_l, t, y, f0, params):
    return _shard_forward(pos_l, t, y, f0, params)


def kernel(t, y, f0, params):
    t = np.asarray(t, np.float32)
    y = np.asarray(y, np.float32)
    f0 = np.asarray(f0, np.float32)
    pos_sh = y[0].reshape(NCORES, LOCAL, 3)
    ydot, scales, translations = _pmapped(pos_sh, t, y, f0, params)
    # every device holds the identical full-shape result; take core 0's copy
    return (np.asarray(ydot[0]), np.asarray(scales[0]),
            np.asarray(translations[0]))
